# revision 91
# baseline (speedup 1.0000x reference)
"""Trainium2 Bass kernel for nn_Decoder_1692217114985 (continuous transpose-conv decoder).

Math (see the reference):
  integ = FF(weights)                         # [B=64, K=400] per-stride integrals
  kval[f,n,k] = MLP_f(grid[n] - center[k])    # masked to the 0.15-window
  out = sigmoid(einsum('fnk,bk->bnf', kval, integ))

Key structural fact: the window is 0.15 wide on a 0.05-spaced 20x20 center
grid, so each grid point has at most ~9 active centers out of 400 (~97%
sparse).  The window mask is a pure function of `grid` (not of the weights),
so the HOST computes the exact fp32 mask and packs only the active
(point, center) pairs for the device:

  - rhs [38, 960]: active-pair local coords, fp16, 3-slice block-diagonal
    packing (6 MLP evals per PE column: 3 pairs x 2 fields), J=10 slots per
    point, G=32 points per (chunk, slice) slot, 3 chunks of W=320 columns.
  - sidx [128, 40] int16: per-point scatter indices (k for field 0,
    512+k for field 1, -1 for inactive slots).

Device flow per core (grid points sharded 256/core, no collectives):
  1. FF MLP transposed (features on partitions) -> integT k-chunks [kc, 64],
     interleaved into the PE gaps of the pair-MLP.
  2. Sparse pair-MLP: 3 chunks x (L1 relu L2 relu L3); L3 outputs stack
     into one PSUM tile [96, 320] via tile_position=(0, 32t), with output
     row 3*f + s so each field's slices are contiguous rows.
  3. Per chunk: copy its 6 valid L3 rows to SBUF (kvs); SBUF->SBUF shuffle
     DMAs stream whole slice-runs (one kvs row per 32 points) into the
     point-major shuf tile [128 n, (nt, f, j)] -- 8 DMAs total, issued as
     each chunk finishes.
  4. gpsimd local_scatter (per-partition indices, negatives ignored, zeroes
     dst): shuf -> kvalDT [128 n, 1024 (f, k)] per n-tile.
  5. PE transposes (identity stationary, 16x [128,128]) flip kvalDT into
     kvalD k-chunks [128 k, 512 (f, nt, n)]; DVE/ACT copy PSUM->SBUF.
     (XBAR DMA-transposes were faster on paper but race their staging
     stores: the scheduler misses those dependency edges on HW.)
  6. Per field: 4 accumulating matmuls integT[kc,64].T @ kvalD[kc,256] ->
     psF, sigmoid, store -- field 0's sigmoid/store overlaps field 1's
     matmuls.

All matmul datapaths fp16 (fp32 PSUM accumulation), masked-out slots never
reach the output (their scatter index is -1), mask boundary handling is
bit-exact with the reference because the host replicates its fp32 ops.

Measured on the 8-core axon setup: ~31 us vs 137 us for the dense baseline
in the same session (the PE runs cold at 1.2 GHz here; it never reaches
the 2.4 GHz warm clock regardless of sustained activity).
"""

import numpy as np
from contextlib import ExitStack

import concourse.bacc as bacc
import concourse.bass as bass
import concourse.tile as tile
from concourse import mybir
from concourse.bass_utils import run_bass_kernel_spmd

F32 = mybir.dt.float32
F16 = mybir.dt.float16
I16 = mybir.dt.int16
AF = mybir.ActivationFunctionType
OP = mybir.AluOpType

B, H, N, F, KH = 64, 256, 2048, 2, 20
K = 400
NCORES = 8
NLOC = N // NCORES          # 256 grid points per core
CHUNKS = [(0, 128), (128, 128), (256, 128), (384, 16)]   # k-chunks of integT
S = 3                        # packed slices per column
# J=9 would suffice (max active is 9) but makes each shuffle DMA write 18B
# per partition at 18B offsets -- not 4B-aligned, and adjacent-field writes
# were observed to corrupt each other intermittently (rel err 0.39 on ~1/3
# of runs).  J=10 keeps every write 4B-aligned and has been rock solid.
J = 10                       # scatter slots per point
G = 32                       # points per (chunk, slice) slot
W = J * G                    # 320 columns per chunk
NCH = 3                      # chunks
FILT = 0.15

# group g (points 32g..32g+31) -> (chunk, slice) slot.  With L3 output rows
# laid out 3*f + s, consecutive-slice groups of one chunk merge into a
# single shuffle DMA per field (4 runs: g0-g2, g3, g4-g5, g6-g7).
SLOT_OF_GROUP = [(g // 3, g % 3) for g in range(8)]
SHUF_RUNS = [(0, [0, 1, 2]), (1, [3]), (1, [4, 5]), (2, [6, 7])]

# big1 f16 column layout: rhs | w1p | w2p | w3p.  (Keep it ONE tensor and
# ONE DMA: a separate small rhs+w1p tensor or a split load both measured
# slower -- extra issue serialization / conservative dual-writer deps.)
RHS0 = 0
W1P0 = NCH * W
W2P0 = W1P0 + 120
W3P0 = W2P0 + 123
BIG1C = W3P0 + 32
# big2 [128, 2336] f16 column layout: wT | ffw1 | ffw2 | ffw3 | identity
WT0, FFW10, FFW20, FFW30, IDN0, BIG2C = 0, 128, 368, 608, 2208, 2336

LAST_RESULTS = None          # BassKernelResults of the most recent run
DEBUG = False                # dump intermediates as extra outputs


def _build_nc():
    nc = bacc.Bacc("TRN2", name="decoder")

    d_big1 = nc.dram_tensor("big1", [128, BIG1C], F16, kind="ExternalInput")
    d_big2 = nc.dram_tensor("big2", [128, BIG2C], F16, kind="ExternalInput")
    d_sidx = nc.dram_tensor("sidx", [128, 2 * 2 * J], I16, kind="ExternalInput")
    d_bias = nc.dram_tensor("bias", [128, 9], F32, kind="ExternalInput")
    d_out = nc.dram_tensor("out", [B, F, NLOC], F32, kind="ExternalOutput")
    if DEBUG:
        d_dkvs = nc.dram_tensor("dkvs", [96, W], F16, kind="ExternalOutput")
        d_dshuf = nc.dram_tensor("dshuf", [128, 4 * J], F16, kind="ExternalOutput")
        d_dkdt = nc.dram_tensor("dkdt", [2, 128, 1024], F16, kind="ExternalOutput")
        d_dkd = nc.dram_tensor("dkd", [4, 128, 512], F16, kind="ExternalOutput")

    with tile.TileContext(nc) as tc, ExitStack() as ctx:
        consts = ctx.enter_context(tc.tile_pool(name="consts", bufs=1))
        persist = ctx.enter_context(tc.tile_pool(name="persist", bufs=1))
        work = ctx.enter_context(tc.tile_pool(name="work", bufs=4))
        psum = ctx.enter_context(tc.tile_pool(name="psum", bufs=1, space="PSUM"))

        # ---- input loads ----
        big1 = consts.tile([128, BIG1C], F16, tag="big1")
        nc.sync.dma_start(out=big1[:], in_=d_big1[:, :])
        # bias/sidx on sync too: this leaves the scalar HWDGE ring entirely
        # unused (both loads still land well before their consumers)
        bias = consts.tile([128, 9], F32, tag="bias")
        nc.sync.dma_start(out=bias[:], in_=d_bias[:, :])
        sidx = consts.tile([128, 4 * J], I16, tag="sidx")
        nc.sync.dma_start(out=sidx[:], in_=d_sidx[:, :])
        big2 = consts.tile([128, BIG2C], F16, tag="big2")
        nc.sync.dma_start(out=big2[:], in_=d_big2[:, :])

        # dummy local_scatter: forces the gpsimd ucode-library reload (and its
        # queue DRAIN) to happen here, overlapped with the input DMAs, instead
        # of on the critical path right before the real scatters
        dumi = consts.tile([16, 2], I16, tag="dumi")
        nc.vector.memset(dumi[:], -1)
        dumd = consts.tile([16, 2], F16, tag="dumd")
        nc.vector.memset(dumd[:], 0.0)
        dumo = consts.tile([16, 2], F16, tag="dumo")
        nc.gpsimd.local_scatter(out_ap=dumo[:], data_ap=dumd[:], idxs_ap=dumi[:],
                                channels=16, num_elems=2, num_idxs=2)

        rhs = big1[:, RHS0:RHS0 + NCH * W]
        w1p = big1[:38, W1P0:W1P0 + 120]
        w2p = big1[:120, W2P0:W2P0 + 123]
        w3p = big1[:123, W3P0:W3P0 + 32]
        b1p = bias[:120, 0:1]
        b2p = bias[:123, 1:2]

        # preload the Sigmoid PWP table while the PE crunches, so the kernel
        # tail doesn't pay the ~1.3us ACT_TABLE_LOAD
        onex = consts.tile([1, 1], F32, tag="onex")
        nc.vector.memset(onex[:], 1.0)
        sigdum = consts.tile([1, 1], F32, tag="sigdum")
        nc.scalar.activation(sigdum[:], onex[:], AF.Sigmoid)

        # ---- sparse pair-MLP: 3 chunks of W columns ----
        kvs = persist.tile([96, W], F16, tag="kvs")
        shuf = persist.tile([128, 4 * J], F16, tag="shuf")

        def emit_run(run, eng):
            t_g, gs = run
            nt, p0 = gs[0] // 4, 32 * (gs[0] % 4)
            s0, ng = SLOT_OF_GROUP[gs[0]][1], len(gs)
            for f in range(F):
                row = 32 * t_g + 3 * f + s0
                eng.dma_start(
                    out=shuf[p0:p0 + 32 * ng,
                             2 * J * nt + f * J:2 * J * nt + (f + 1) * J],
                    in_=kvs[row:row + ng, :])

        def emit_gathers(ch):
            # SBUF->SBUF shuffle for the slot-runs living in chunk `ch`: kvs
            # rows 32*t + 3*f + s (each laid out (m, j) row-major) stream
            # straight into runs of 32 partitions x J cols of shuf.  A run of
            # consecutive slices is one DMA: src rows iterate s, dst
            # partitions iterate (s, m) -- same order.  Mostly on the sync
            # ring; the g6-g7 pair (the scatter1 gate, ready last) goes to
            # scalar so it isn't queued behind six earlier sync issues.
            # (gpsimd issues lengthen the inter-scatter DRAIN -- avoid.)
            # All on the sync ring.  Measured alternatives all regress:
            # scalar issues interfere with the relu/FF activations, gpsimd
            # issues lengthen the inter-scatter DRAIN, and even moving just
            # the late g6-g7 pair to scalar doesn't move the finish line --
            # the PE transpose+matmul tail is equally gating.
            # the g6-g7 pair (the scatter1 gate, ready last) goes on the
            # now-otherwise-empty scalar ring so it issues as soon as its
            # data is ready instead of queueing behind six sync issues
            for run in SHUF_RUNS:
                if run[0] != ch:
                    continue
                emit_run(run, nc.scalar if run[1] == [6, 7] else nc.sync)

        ps3 = psum.tile([96, W], F32, tag="ps3", name="ps3")
        for ch in range(NCH):
            csl = slice(ch * W, (ch + 1) * W)
            ps1 = psum.tile([120, W], F32, tag="ps1", bufs=2)
            r = 32 * (ch % 2)   # dual 6-row strips so consecutive L1s overlap
            nc.tensor.matmul(ps1[:], big1[r:r + 6, W1P0:W1P0 + 120],
                             big1[r:r + 6, RHS0 + ch * W:RHS0 + (ch + 1) * W],
                             start=True, stop=True, tile_position=(r, 0))
            h1 = work.tile([120, W], F16, tag="h1")
            if ch % 2 == 0:
                nc.scalar.activation(h1[:], ps1[:], AF.Relu, bias=b1p)
            else:
                nc.vector.tensor_scalar(h1[:], ps1[:], b1p, 0.0, OP.add, OP.max)
            ps2 = psum.tile([123, W], F32, tag="ps2", bufs=2)
            nc.tensor.matmul(ps2[:], w2p, h1[:], start=True, stop=True)
            h2 = work.tile([123, W], F16, tag="h2")
            if ch % 2 == 1:
                nc.scalar.activation(h2[:], ps2[:], AF.Relu, bias=b2p)
            else:
                nc.vector.tensor_scalar(h2[:], ps2[:], b2p, 0.0, OP.add, OP.max)
            nc.tensor.matmul(ps3[32 * ch:32 * ch + 32, :], w3p, h2[:],
                             start=True, stop=True, tile_position=(0, 32 * ch))
            # copy this chunk's 6 valid L3 rows (3f+s) to SBUF so its
            # shuffle DMAs can start before the whole MLP finishes
            if ch % 2 == 0:
                nc.vector.tensor_copy(kvs[32 * ch:32 * ch + 6, :],
                                      ps3[32 * ch:32 * ch + 6, :])
            else:
                nc.scalar.activation(kvs[32 * ch:32 * ch + 6, :],
                                     ps3[32 * ch:32 * ch + 6, :], AF.Identity)
            emit_gathers(ch)

        # ---- local_scatter -> kvalDT [n, (f, k)] ----
        # (one full-tile call per ntile: partition-offset slices with
        # channels<128 leave the upper cores inactive on HW -> garbage)
        kvalDT = [persist.tile([128, 1024], F16, tag=f"kvalDT{nt}",
                               name=f"kvalDT{nt}") for nt in range(2)]
        for nt in range(2):
            nc.gpsimd.local_scatter(
                out_ap=kvalDT[nt][:],
                data_ap=shuf[:, 2 * J * nt:2 * J * (nt + 1)],
                idxs_ap=sidx[:, 2 * J * nt:2 * J * (nt + 1)],
                channels=128, num_elems=1024, num_idxs=2 * J)

        # ---- FF MLP (transposed): integT chunks [kc, 64] ----
        ffb1c = bias[:120, 2:3]
        # FF psums alternate between the psff bank and ps2's (dead after the
        # MLP): a single bank made every L3 matmul-pair wait ~0.7us for the
        # previous identity-activation to drain it (seen as w=666 stalls)
        ff_tags = ["psff", "ps2"]
        ff_ctr = [0]

        def ff_ps():
            tag = ff_tags[ff_ctr[0] % 2]
            ff_ctr[0] += 1
            return psum.tile([128, B], F32, tag=tag,
                             bufs=1 if tag == "psff" else 2, name="ps")

        ps = ff_ps()
        nc.tensor.matmul(ps[:120, :], big2[:, FFW10:FFW10 + 120],
                         big2[:, WT0:WT0 + 64], start=True, stop=False)
        nc.tensor.matmul(ps[:120, :], big2[:, FFW10 + 120:FFW10 + 240],
                         big2[:, WT0 + 64:WT0 + 128], start=False, stop=True)
        h1ff = work.tile([120, B], F16, tag="h1ff")
        nc.scalar.activation(h1ff[:], ps[:120, :], AF.Tanh, bias=ffb1c)
        h2ffa = work.tile([120, B], F16, tag="h2ffa")
        h2ffb = work.tile([120, B], F16, tag="h2ffb")
        for m, h2ff in enumerate((h2ffa, h2ffb)):
            ps = ff_ps()
            nc.tensor.matmul(ps[:120, :],
                             big2[:120, FFW20 + 120 * m:FFW20 + 120 * (m + 1)],
                             h1ff[:], start=True, stop=True)
            nc.scalar.activation(h2ff[:], ps[:120, :], AF.Tanh,
                                 bias=bias[:120, 3 + m:4 + m])
        integT = []
        for ci, (k0, kc) in enumerate(CHUNKS):
            ps = ff_ps()
            nc.tensor.matmul(ps[:kc, :], big2[:120, FFW30 + k0:FFW30 + k0 + kc],
                             h2ffa[:], start=True, stop=False)
            nc.tensor.matmul(ps[:kc, :],
                             big2[:120, FFW30 + 800 + k0:FFW30 + 800 + k0 + kc],
                             h2ffb[:], start=False, stop=True)
            it = persist.tile([128, B], F16, tag=f"integT{ci}")
            nc.scalar.activation(it[:kc, :], ps[:kc, :], AF.Identity,
                                 bias=bias[:kc, 5 + ci:6 + ci])
            integT.append(it)

        # ---- XBAR DMA-transposes: dstag rows (f, nt, p) -> kvalD [k, (f,n)] ----
        # ---- PE transposes kvalDT -> kvalD k-chunks [k, (nt, f, n)] ----
        # DMA/XBAR transposes raced their staging stores (the scheduler
        # misses those dep edges), so the transpose runs on the PE instead:
        # regular, fully dep-tracked instructions, and the PE is idle in
        # the tail anyway.  The identity stationary is loaded once.
        ident = big2[:, IDN0:IDN0 + 128]
        outsb = persist.tile([B, F, NLOC], F32, tag="outsb")
        kvalD = [persist.tile([128, 512], F16, tag=f"kvalD{c}", name=f"kvalD{c}")
                 for c in range(4)]

        # transpose PSUM rotates over 4 banks: ps1's two plus the ps3 and
        # psff banks (both dead by transpose time) -- depth-4 pipelining so
        # the PE doesn't stall on PSUM->SBUF copy completion
        tp_tags = ["ps1", "ps3", "ps1", "psff"]
        tp_ctr = [0]

        def emit_transposes(nt, f):
            # kvalD columns are (f, nt, p) so per-field matmuls slice
            # contiguous 256-column blocks.  (Merging the two fields'
            # chunk-3 pieces into one strided-AP transpose passes CoreSim
            # but fails NEFF compilation -- keep the plain 4-per-(nt,f).)
            for ci in range(4):
                tag = tp_tags[tp_ctr[0] % 4]
                tp_ctr[0] += 1
                pst = psum.tile([128, 128], F16, tag=tag,
                                bufs=2 if tag == "ps1" else 1)
                nc.tensor.transpose(
                    pst[:], kvalDT[nt][:, 512 * f + 128 * ci:512 * f + 128 * (ci + 1)],
                    ident)
                dsl = kvalD[ci][:, 256 * f + 128 * nt:256 * f + 128 * (nt + 1)]
                # all copies on vector: it is otherwise idle in the tail,
                # while scalar is still draining FF identities when the
                # transpose stream starts
                nc.vector.tensor_copy(dsl, pst[:])

        # nt0 transposes can run (behind FF on the PE) while ntile-1's
        # shuffle/scatter chain is still in flight; field 0's matmul chain
        # fires before ntile-1's field-1 transposes so the sigmoid/store of
        # field 0 overlaps the rest of the tail
        psFs = []
        for f in range(F):
            psFs.append(psum.tile([B, 256], F32, tag="psf", bufs=2, name=f"psF{f}"))
        emit_transposes(0, 0)
        emit_transposes(0, 1)
        emit_transposes(1, 0)
        for ci, (k0, kc) in enumerate(CHUNKS):
            nc.tensor.matmul(psFs[0][:], integT[ci][:kc, :],
                             kvalD[ci][:kc, 0:256],
                             start=(ci == 0), stop=(ci == 3))
        emit_transposes(1, 1)
        for ci, (k0, kc) in enumerate(CHUNKS):
            nc.tensor.matmul(psFs[1][:], integT[ci][:kc, :],
                             kvalD[ci][:kc, 256:512],
                             start=(ci == 0), stop=(ci == 3))
        # per-field sigmoid + store, field 0 pipelined ahead of field 1
        for f in range(F):
            nc.scalar.activation(outsb[:, f, :], psFs[f][:], AF.Sigmoid)
            nc.sync.dma_start(out=d_out[:, f, :], in_=outsb[:, f, :])
        if DEBUG:
            nc.sync.dma_start(out=d_dkvs[:, :], in_=kvs[:])
            nc.sync.dma_start(out=d_dshuf[:, :], in_=shuf[:])
            for nt in range(2):
                nc.scalar.dma_start(out=d_dkdt[nt, :, :], in_=kvalDT[nt][:])
            for c in range(4):
                nc.scalar.dma_start(out=d_dkd[c, :, :], in_=kvalD[c][:])

    nc.finalize()
    return nc


_NC_CACHE = None


def _get_nc():
    global _NC_CACHE
    if _NC_CACHE is None:
        _NC_CACHE = _build_nc()
    return _NC_CACHE


def _pack_shared(w):
    """Weight packing shared across cores (pure reshuffling)."""
    f32, f16 = np.float32, np.float16
    k_w1, k_b1 = w["k_w1"].astype(f32), w["k_b1"].astype(f32)
    k_w2, k_b2 = w["k_w2"].astype(f32), w["k_b2"].astype(f32)
    k_w3, k_b3 = w["k_w3"].astype(f32), w["k_b3"].astype(f32)
    w1p = np.zeros((38, 120), f32)
    b1p = np.zeros((120,), f32)
    w2p = np.zeros((120, 123), f32)
    b2p = np.zeros((123,), f32)
    w3p = np.zeros((123, 32), f32)
    for s in range(S):
        for f in range(F):
            o = s * 40 + f * 20
            for d in range(2):
                w1p[2 * s + d, o:o + 20] = k_w1[f, d]
                w1p[32 + 2 * s + d, o:o + 20] = k_w1[f, d]
            b1p[o:o + 20] = k_b1[f]
            w2p[o:o + 20, s * 41 + f * 20:s * 41 + f * 20 + 20] = k_w2[f]
            b2p[s * 41 + f * 20:s * 41 + f * 20 + 20] = k_b2[f]
            # L3 output row = 3*f + s so consecutive-slice shuffle runs are
            # contiguous kvs rows per field
            w3p[s * 41 + f * 20:s * 41 + f * 20 + 20, 3 * f + s] = k_w3[f, :, 0]
            w3p[s * 41 + 40, 3 * f + s] = k_b3[f, 0]
        b2p[s * 41 + 40] = 1.0

    big2 = np.zeros((128, BIG2C), f16)
    wT = np.ascontiguousarray(w["weights"].astype(f32).T).astype(f16)  # [256,64]
    big2[:, WT0:WT0 + 64] = wT[:128]
    big2[:, WT0 + 64:WT0 + 128] = wT[128:]
    ffw1 = w["ff_w1"].astype(f16)            # [256, 120]
    big2[:, FFW10:FFW10 + 120] = ffw1[:128]
    big2[:, FFW10 + 120:FFW10 + 240] = ffw1[128:]
    big2[:120, FFW20:FFW20 + 240] = w["ff_w2"].astype(f16)
    ffw3 = w["ff_w3"].astype(f16)            # [240, 400]
    big2[:120, FFW30:FFW30 + 400] = ffw3[:120, :]
    big2[:120, FFW30 + 800:FFW30 + 1200] = ffw3[120:, :]
    big2[:, IDN0:IDN0 + 128] = np.eye(128, dtype=f16)

    bias = np.zeros((128, 9), f32)
    bias[:120, 0] = b1p
    bias[:123, 1] = b2p
    bias[:120, 2] = w["ff_b1"].astype(f32)
    bias[:120, 3] = w["ff_b2"].astype(f32)[:120]
    bias[:120, 4] = w["ff_b2"].astype(f32)[120:240]
    ffb3 = np.zeros((512,), f32)
    ffb3[:K] = w["ff_b3"].astype(f32)
    for ci in range(4):
        bias[:, 5 + ci] = ffb3[128 * ci:128 * (ci + 1)]

    return w1p.astype(f16), w2p.astype(f16), w3p.astype(f16), big2, bias


def _pack_core(grid_c, cx, cy, inside_c):
    """Per-core rhs + scatter-index packing from the exact host mask."""
    f16 = np.float16
    rhs = np.full((38, NCH * W), 0.075, np.float32)
    sidx = np.full((128, 4 * J), -1, np.int16)
    for n in range(NLOC):
        ks = np.nonzero(inside_c[n])[0]
        assert len(ks) <= J
        g = n // G
        t_g, s_g = SLOT_OF_GROUP[g]
        m = n % G
        nt, prt = n // 128, n % 128
        cols = t_g * W + m * J + np.arange(len(ks))
        rhs[2 * s_g + 0, cols] = grid_c[n, 0] - cx[ks]   # exact fp32
        rhs[2 * s_g + 1, cols] = grid_c[n, 1] - cy[ks]
        base = 2 * J * nt
        sidx[prt, base:base + len(ks)] = ks              # field 0 -> col k
        sidx[prt, base + J:base + J + len(ks)] = 512 + ks  # field 1
    rhs[32:38, :] = rhs[0:6, :]   # dual L1 strip
    return rhs.astype(f16), sidx


def kernel(**inputs):
    global LAST_RESULTS
    nc = _get_nc()
    f32 = np.float32
    w1p, w2p, w3p, big2, bias = _pack_shared(inputs)

    grid = inputs["grid"].astype(f32)
    g1 = (np.arange(20, dtype=f32) * f32(0.05)).astype(f32)
    cx, cy = np.repeat(g1, 20), np.tile(g1, 20)
    centers = np.stack([cx, cy], -1)
    local = grid[:, None, :] - centers[None, :, :]
    inside = ((local >= 0) & (local <= f32(FILT))).all(-1)   # exact fp32 mask

    in_maps = []
    for c in range(NCORES):
        rhs, sidx = _pack_core(grid[c * NLOC:(c + 1) * NLOC], cx, cy,
                               inside[c * NLOC:(c + 1) * NLOC])
        big1 = np.zeros((128, BIG1C), np.float16)
        big1[:38, RHS0:RHS0 + NCH * W] = rhs
        big1[:38, W1P0:W1P0 + 120] = w1p
        big1[:120, W2P0:W2P0 + 123] = w2p
        big1[:123, W3P0:W3P0 + 32] = w3p
        in_maps.append(dict(big1=big1, big2=big2, sidx=sidx, bias=bias))

    res = run_bass_kernel_spmd(nc, in_maps, core_ids=list(range(NCORES)))
    LAST_RESULTS = res
    # device out is [B, F, NLOC]; harness wants [B, N, F]
    out = np.concatenate([r["out"].transpose(0, 2, 1) for r in res.results],
                         axis=1)
    return out


# revision 92
# speedup vs baseline: 1.0165x; 1.0165x over previous
"""Trainium2 Bass kernel for nn_Decoder_1692217114985 (continuous transpose-conv decoder).

Math (see the reference):
  integ = FF(weights)                         # [B=64, K=400] per-stride integrals
  kval[f,n,k] = MLP_f(grid[n] - center[k])    # masked to the 0.15-window
  out = sigmoid(einsum('fnk,bk->bnf', kval, integ))

Key structural fact: the window is 0.15 wide on a 0.05-spaced 20x20 center
grid, so each grid point has at most ~9 active centers out of 400 (~97%
sparse).  The window mask is a pure function of `grid` (not of the weights),
so the HOST computes the exact fp32 mask and packs only the active
(point, center) pairs for the device:

  - rhs [38, 960]: active-pair local coords, fp16, 3-slice block-diagonal
    packing (6 MLP evals per PE column: 3 pairs x 2 fields), J=10 slots per
    point, G=32 points per (chunk, slice) slot, 3 chunks of W=320 columns.
  - sidx [128, 40] int16: per-point scatter indices (k for field 0,
    512+k for field 1, -1 for inactive slots).

Device flow per core (grid points sharded 256/core, no collectives):
  1. FF MLP transposed (features on partitions) -> integT k-chunks [kc, 64],
     interleaved into the PE gaps of the pair-MLP.
  2. Sparse pair-MLP: 3 chunks x (L1 relu L2 relu L3); L3 outputs stack
     into one PSUM tile [96, 320] via tile_position=(0, 32t), with output
     row 3*f + s so each field's slices are contiguous rows.
  3. Per chunk: copy its 6 valid L3 rows to SBUF (kvs); SBUF->SBUF shuffle
     DMAs stream whole slice-runs (one kvs row per 32 points) into the
     point-major shuf tile [128 n, (nt, f, j)] -- 8 DMAs total, issued as
     each chunk finishes.
  4. gpsimd local_scatter (per-partition indices, negatives ignored, zeroes
     dst): shuf -> kvalDT [128 n, 1024 (f, k)] per n-tile.
  5. PE transposes (identity stationary, 16x [128,128]) flip kvalDT into
     kvalD k-chunks [128 k, 512 (f, nt, n)]; DVE/ACT copy PSUM->SBUF.
     (XBAR DMA-transposes were faster on paper but race their staging
     stores: the scheduler misses those dependency edges on HW.)
  6. Per field: 4 accumulating matmuls integT[kc,64].T @ kvalD[kc,256] ->
     psF, sigmoid, store -- field 0's sigmoid/store overlaps field 1's
     matmuls.

All matmul datapaths fp16 (fp32 PSUM accumulation), masked-out slots never
reach the output (their scatter index is -1), mask boundary handling is
bit-exact with the reference because the host replicates its fp32 ops.

Measured on the 8-core axon setup: ~31 us vs 137 us for the dense baseline
in the same session (the PE runs cold at 1.2 GHz here; it never reaches
the 2.4 GHz warm clock regardless of sustained activity).
"""

import numpy as np
from contextlib import ExitStack

import concourse.bacc as bacc
import concourse.bass as bass
import concourse.tile as tile
from concourse import mybir
from concourse.bass_utils import run_bass_kernel_spmd

F32 = mybir.dt.float32
F16 = mybir.dt.float16
I16 = mybir.dt.int16
AF = mybir.ActivationFunctionType
OP = mybir.AluOpType

B, H, N, F, KH = 64, 256, 2048, 2, 20
K = 400
NCORES = 8
NLOC = N // NCORES          # 256 grid points per core
CHUNKS = [(0, 128), (128, 128), (256, 128), (384, 16)]   # k-chunks of integT
S = 3                        # packed slices per column
# J=9 would suffice (max active is 9) but makes each shuffle DMA write 18B
# per partition at 18B offsets -- not 4B-aligned, and adjacent-field writes
# were observed to corrupt each other intermittently (rel err 0.39 on ~1/3
# of runs).  J=10 keeps every write 4B-aligned and has been rock solid.
J = 10                       # scatter slots per point
G = 32                       # points per (chunk, slice) slot
W = J * G                    # 320 columns per chunk
NCH = 3                      # chunks
FILT = 0.15

# group g (points 32g..32g+31) -> (chunk, slice) slot.  With L3 output rows
# laid out 3*f + s, consecutive-slice groups of one chunk merge into a
# single shuffle DMA per field (4 runs: g0-g2, g3, g4-g5, g6-g7).
SLOT_OF_GROUP = [(g // 3, g % 3) for g in range(8)]
SHUF_RUNS = [(0, [0, 1, 2]), (1, [3]), (1, [4, 5]), (2, [6, 7])]

# big1 f16 column layout: rhs | w1p | w2p | w3p.  (Keep it ONE tensor and
# ONE DMA: a separate small rhs+w1p tensor or a split load both measured
# slower -- extra issue serialization / conservative dual-writer deps.)
RHS0 = 0
W1P0 = NCH * W
W2P0 = W1P0 + 120
W3P0 = W2P0 + 123
BIG1C = W3P0 + 32
# big2 [128, 2336] f16 column layout: wT | ffw1 | ffw2 | ffw3 | identity
WT0, FFW10, FFW20, FFW30, IDN0, BIG2C = 0, 128, 368, 608, 2208, 2336

LAST_RESULTS = None          # BassKernelResults of the most recent run
DEBUG = False                # dump intermediates as extra outputs


def _build_nc():
    nc = bacc.Bacc("TRN2", name="decoder")

    d_big1 = nc.dram_tensor("big1", [128, BIG1C], F16, kind="ExternalInput")
    d_big2 = nc.dram_tensor("big2", [128, BIG2C], F16, kind="ExternalInput")
    d_sidx = nc.dram_tensor("sidx", [128, 2 * 2 * J], I16, kind="ExternalInput")
    d_bias = nc.dram_tensor("bias", [128, 9], F32, kind="ExternalInput")
    d_out = nc.dram_tensor("out", [B, F, NLOC], F32, kind="ExternalOutput")
    if DEBUG:
        d_dkvs = nc.dram_tensor("dkvs", [96, W], F16, kind="ExternalOutput")
        d_dshuf = nc.dram_tensor("dshuf", [128, 4 * J], F16, kind="ExternalOutput")
        d_dkdt = nc.dram_tensor("dkdt", [2, 128, 1024], F16, kind="ExternalOutput")
        d_dkd = nc.dram_tensor("dkd", [4, 128, 512], F16, kind="ExternalOutput")

    with tile.TileContext(nc) as tc, ExitStack() as ctx:
        consts = ctx.enter_context(tc.tile_pool(name="consts", bufs=1))
        persist = ctx.enter_context(tc.tile_pool(name="persist", bufs=1))
        work = ctx.enter_context(tc.tile_pool(name="work", bufs=4))
        psum = ctx.enter_context(tc.tile_pool(name="psum", bufs=1, space="PSUM"))

        # ---- input loads ----
        big1 = consts.tile([128, BIG1C], F16, tag="big1")
        nc.sync.dma_start(out=big1[:], in_=d_big1[:, :])
        # bias/sidx on sync too: this leaves the scalar HWDGE ring entirely
        # unused (both loads still land well before their consumers)
        bias = consts.tile([128, 9], F32, tag="bias")
        nc.sync.dma_start(out=bias[:], in_=d_bias[:, :])
        sidx = consts.tile([128, 4 * J], I16, tag="sidx")
        nc.sync.dma_start(out=sidx[:], in_=d_sidx[:, :])
        big2 = consts.tile([128, BIG2C], F16, tag="big2")
        nc.sync.dma_start(out=big2[:], in_=d_big2[:, :])

        # dummy local_scatter: forces the gpsimd ucode-library reload (and its
        # queue DRAIN) to happen here, overlapped with the input DMAs, instead
        # of on the critical path right before the real scatters
        dumi = consts.tile([16, 2], I16, tag="dumi")
        nc.vector.memset(dumi[:], -1)
        dumd = consts.tile([16, 2], F16, tag="dumd")
        nc.vector.memset(dumd[:], 0.0)
        dumo = consts.tile([16, 2], F16, tag="dumo")
        nc.gpsimd.local_scatter(out_ap=dumo[:], data_ap=dumd[:], idxs_ap=dumi[:],
                                channels=16, num_elems=2, num_idxs=2)

        rhs = big1[:, RHS0:RHS0 + NCH * W]
        w1p = big1[:38, W1P0:W1P0 + 120]
        w2p = big1[:120, W2P0:W2P0 + 123]
        w3p = big1[:123, W3P0:W3P0 + 32]
        b1p = bias[:120, 0:1]
        b2p = bias[:123, 1:2]

        # preload the Sigmoid PWP table while the PE crunches, so the kernel
        # tail doesn't pay the ~1.3us ACT_TABLE_LOAD
        onex = consts.tile([1, 1], F32, tag="onex")
        nc.vector.memset(onex[:], 1.0)
        sigdum = consts.tile([1, 1], F32, tag="sigdum")
        nc.scalar.activation(sigdum[:], onex[:], AF.Sigmoid)

        # ---- sparse pair-MLP: 3 chunks of W columns ----
        kvs = persist.tile([96, W], F16, tag="kvs")
        shuf = persist.tile([128, 4 * J], F16, tag="shuf")

        def emit_run(run, eng):
            t_g, gs = run
            nt, p0 = gs[0] // 4, 32 * (gs[0] % 4)
            s0, ng = SLOT_OF_GROUP[gs[0]][1], len(gs)
            for f in range(F):
                row = 32 * t_g + 3 * f + s0
                eng.dma_start(
                    out=shuf[p0:p0 + 32 * ng,
                             2 * J * nt + f * J:2 * J * nt + (f + 1) * J],
                    in_=kvs[row:row + ng, :])

        def emit_gathers(ch):
            # SBUF->SBUF shuffle for the slot-runs living in chunk `ch`: kvs
            # rows 32*t + 3*f + s (each laid out (m, j) row-major) stream
            # straight into runs of 32 partitions x J cols of shuf.  A run of
            # consecutive slices is one DMA: src rows iterate s, dst
            # partitions iterate (s, m) -- same order.  Mostly on the sync
            # ring; the g6-g7 pair (the scatter1 gate, ready last) goes to
            # scalar so it isn't queued behind six earlier sync issues.
            # (gpsimd issues lengthen the inter-scatter DRAIN -- avoid.)
            # All on the sync ring.  Measured alternatives all regress:
            # scalar issues interfere with the relu/FF activations, gpsimd
            # issues lengthen the inter-scatter DRAIN, and even moving just
            # the late g6-g7 pair to scalar doesn't move the finish line --
            # the PE transpose+matmul tail is equally gating.
            # the g6-g7 pair (the scatter1 gate, ready last) goes on the
            # now-otherwise-empty scalar ring so it issues as soon as its
            # data is ready instead of queueing behind six sync issues
            for run in SHUF_RUNS:
                if run[0] != ch:
                    continue
                emit_run(run, nc.scalar if run[1] == [6, 7] else nc.sync)

        ps3 = psum.tile([96, W], F32, tag="ps3", name="ps3")
        for ch in range(NCH):
            csl = slice(ch * W, (ch + 1) * W)
            ps1 = psum.tile([120, W], F32, tag="ps1", bufs=2)
            r = 32 * (ch % 2)   # dual 6-row strips so consecutive L1s overlap
            nc.tensor.matmul(ps1[:], big1[r:r + 6, W1P0:W1P0 + 120],
                             big1[r:r + 6, RHS0 + ch * W:RHS0 + (ch + 1) * W],
                             start=True, stop=True, tile_position=(r, 0))
            h1 = work.tile([120, W], F16, tag="h1")
            if ch % 2 == 0:
                nc.scalar.activation(h1[:], ps1[:], AF.Relu, bias=b1p)
            else:
                nc.vector.tensor_scalar(h1[:], ps1[:], b1p, 0.0, OP.add, OP.max)
            ps2 = psum.tile([123, W], F32, tag="ps2", bufs=2)
            nc.tensor.matmul(ps2[:], w2p, h1[:], start=True, stop=True)
            h2 = work.tile([123, W], F16, tag="h2")
            if ch % 2 == 1:
                nc.scalar.activation(h2[:], ps2[:], AF.Relu, bias=b2p)
            else:
                nc.vector.tensor_scalar(h2[:], ps2[:], b2p, 0.0, OP.add, OP.max)
            nc.tensor.matmul(ps3[32 * ch:32 * ch + 32, :], w3p, h2[:],
                             start=True, stop=True, tile_position=(0, 32 * ch))
            # copy this chunk's 6 valid L3 rows (3f+s) to SBUF so its
            # shuffle DMAs can start before the whole MLP finishes
            if ch % 2 == 0:
                nc.vector.tensor_copy(kvs[32 * ch:32 * ch + 6, :],
                                      ps3[32 * ch:32 * ch + 6, :])
            else:
                nc.scalar.activation(kvs[32 * ch:32 * ch + 6, :],
                                     ps3[32 * ch:32 * ch + 6, :], AF.Identity)
            emit_gathers(ch)

        # ---- local_scatter -> kvalDT [n, (f, k)] ----
        # (one full-tile call per ntile: partition-offset slices with
        # channels<128 leave the upper cores inactive on HW -> garbage)
        kvalDT = [persist.tile([128, 1024], F16, tag=f"kvalDT{nt}",
                               name=f"kvalDT{nt}") for nt in range(2)]
        for nt in range(2):
            nc.gpsimd.local_scatter(
                out_ap=kvalDT[nt][:],
                data_ap=shuf[:, 2 * J * nt:2 * J * (nt + 1)],
                idxs_ap=sidx[:, 2 * J * nt:2 * J * (nt + 1)],
                channels=128, num_elems=1024, num_idxs=2 * J)

        # ---- FF MLP (transposed): integT chunks [kc, 64] ----
        ffb1c = bias[:120, 2:3]
        # FF psums alternate between the psff bank and ps2's (dead after the
        # MLP): a single bank made every L3 matmul-pair wait ~0.7us for the
        # previous identity-activation to drain it (seen as w=666 stalls)
        ff_tags = ["psff", "ps2"]
        ff_ctr = [0]

        def ff_ps():
            tag = ff_tags[ff_ctr[0] % 2]
            ff_ctr[0] += 1
            return psum.tile([128, B], F32, tag=tag,
                             bufs=1 if tag == "psff" else 2, name="ps")

        ps = ff_ps()
        nc.tensor.matmul(ps[:120, :], big2[:, FFW10:FFW10 + 120],
                         big2[:, WT0:WT0 + 64], start=True, stop=False)
        nc.tensor.matmul(ps[:120, :], big2[:, FFW10 + 120:FFW10 + 240],
                         big2[:, WT0 + 64:WT0 + 128], start=False, stop=True)
        h1ff = work.tile([120, B], F16, tag="h1ff")
        nc.scalar.activation(h1ff[:], ps[:120, :], AF.Tanh, bias=ffb1c)
        h2ffa = work.tile([120, B], F16, tag="h2ffa")
        h2ffb = work.tile([120, B], F16, tag="h2ffb")
        for m, h2ff in enumerate((h2ffa, h2ffb)):
            ps = ff_ps()
            nc.tensor.matmul(ps[:120, :],
                             big2[:120, FFW20 + 120 * m:FFW20 + 120 * (m + 1)],
                             h1ff[:], start=True, stop=True)
            nc.scalar.activation(h2ff[:], ps[:120, :], AF.Tanh,
                                 bias=bias[:120, 3 + m:4 + m])
        integT = []
        for ci, (k0, kc) in enumerate(CHUNKS):
            ps = ff_ps()
            nc.tensor.matmul(ps[:kc, :], big2[:120, FFW30 + k0:FFW30 + k0 + kc],
                             h2ffa[:], start=True, stop=False)
            nc.tensor.matmul(ps[:kc, :],
                             big2[:120, FFW30 + 800 + k0:FFW30 + 800 + k0 + kc],
                             h2ffb[:], start=False, stop=True)
            it = persist.tile([128, B], F16, tag=f"integT{ci}")
            nc.scalar.activation(it[:kc, :], ps[:kc, :], AF.Identity,
                                 bias=bias[:kc, 5 + ci:6 + ci])
            integT.append(it)

        # ---- XBAR DMA-transposes: dstag rows (f, nt, p) -> kvalD [k, (f,n)] ----
        # ---- PE transposes kvalDT -> kvalD k-chunks [k, (nt, f, n)] ----
        # DMA/XBAR transposes raced their staging stores (the scheduler
        # misses those dep edges), so the transpose runs on the PE instead:
        # regular, fully dep-tracked instructions, and the PE is idle in
        # the tail anyway.  The identity stationary is loaded once.
        ident = big2[:, IDN0:IDN0 + 128]
        outsb = persist.tile([B, F, NLOC], F32, tag="outsb")
        kvalD = [persist.tile([128, 512], F16, tag=f"kvalD{c}", name=f"kvalD{c}")
                 for c in range(4)]

        # transpose PSUM rotates over 4 banks: ps1's two plus the ps3 and
        # psff banks (both dead by transpose time) -- depth-4 pipelining so
        # the PE doesn't stall on PSUM->SBUF copy completion
        tp_tags = ["ps1", "ps3", "ps1", "psff"]
        tp_ctr = [0]

        def emit_transposes(nt, f):
            # kvalD columns are (f, nt, p) so per-field matmuls slice
            # contiguous 256-column blocks.  (Merging the two fields'
            # chunk-3 pieces into one strided-AP transpose passes CoreSim
            # but fails NEFF compilation -- keep the plain 4-per-(nt,f).)
            for ci in range(4):
                tag = tp_tags[tp_ctr[0] % 4]
                tp_ctr[0] += 1
                pst = psum.tile([128, 128], F16, tag=tag,
                                bufs=2 if tag == "ps1" else 1)
                nc.tensor.transpose(
                    pst[:], kvalDT[nt][:, 512 * f + 128 * ci:512 * f + 128 * (ci + 1)],
                    ident)
                dsl = kvalD[ci][:, 256 * f + 128 * nt:256 * f + 128 * (nt + 1)]
                # copies alternate vector/scalar: all-on-vector measured
                # slightly slower (copy-stream serialization outweighs the
                # scalar/FF-identity contention it avoids)
                if (nt + f + ci) % 2 == 0:
                    nc.vector.tensor_copy(dsl, pst[:])
                else:
                    nc.scalar.activation(dsl, pst[:], AF.Identity)

        # nt0 transposes can run (behind FF on the PE) while ntile-1's
        # shuffle/scatter chain is still in flight; field 0's matmul chain
        # fires before ntile-1's field-1 transposes so the sigmoid/store of
        # field 0 overlaps the rest of the tail
        psFs = []
        for f in range(F):
            psFs.append(psum.tile([B, 256], F32, tag="psf", bufs=2, name=f"psF{f}"))
        emit_transposes(0, 0)
        emit_transposes(0, 1)
        emit_transposes(1, 0)
        for ci, (k0, kc) in enumerate(CHUNKS):
            nc.tensor.matmul(psFs[0][:], integT[ci][:kc, :],
                             kvalD[ci][:kc, 0:256],
                             start=(ci == 0), stop=(ci == 3))
        emit_transposes(1, 1)
        for ci, (k0, kc) in enumerate(CHUNKS):
            nc.tensor.matmul(psFs[1][:], integT[ci][:kc, :],
                             kvalD[ci][:kc, 256:512],
                             start=(ci == 0), stop=(ci == 3))
        # per-field sigmoid + store, field 0 pipelined ahead of field 1
        for f in range(F):
            nc.scalar.activation(outsb[:, f, :], psFs[f][:], AF.Sigmoid)
            nc.sync.dma_start(out=d_out[:, f, :], in_=outsb[:, f, :])
        if DEBUG:
            nc.sync.dma_start(out=d_dkvs[:, :], in_=kvs[:])
            nc.sync.dma_start(out=d_dshuf[:, :], in_=shuf[:])
            for nt in range(2):
                nc.scalar.dma_start(out=d_dkdt[nt, :, :], in_=kvalDT[nt][:])
            for c in range(4):
                nc.scalar.dma_start(out=d_dkd[c, :, :], in_=kvalD[c][:])

    nc.finalize()
    return nc


_NC_CACHE = None


def _get_nc():
    global _NC_CACHE
    if _NC_CACHE is None:
        _NC_CACHE = _build_nc()
    return _NC_CACHE


def _pack_shared(w):
    """Weight packing shared across cores (pure reshuffling)."""
    f32, f16 = np.float32, np.float16
    k_w1, k_b1 = w["k_w1"].astype(f32), w["k_b1"].astype(f32)
    k_w2, k_b2 = w["k_w2"].astype(f32), w["k_b2"].astype(f32)
    k_w3, k_b3 = w["k_w3"].astype(f32), w["k_b3"].astype(f32)
    w1p = np.zeros((38, 120), f32)
    b1p = np.zeros((120,), f32)
    w2p = np.zeros((120, 123), f32)
    b2p = np.zeros((123,), f32)
    w3p = np.zeros((123, 32), f32)
    for s in range(S):
        for f in range(F):
            o = s * 40 + f * 20
            for d in range(2):
                w1p[2 * s + d, o:o + 20] = k_w1[f, d]
                w1p[32 + 2 * s + d, o:o + 20] = k_w1[f, d]
            b1p[o:o + 20] = k_b1[f]
            w2p[o:o + 20, s * 41 + f * 20:s * 41 + f * 20 + 20] = k_w2[f]
            b2p[s * 41 + f * 20:s * 41 + f * 20 + 20] = k_b2[f]
            # L3 output row = 3*f + s so consecutive-slice shuffle runs are
            # contiguous kvs rows per field
            w3p[s * 41 + f * 20:s * 41 + f * 20 + 20, 3 * f + s] = k_w3[f, :, 0]
            w3p[s * 41 + 40, 3 * f + s] = k_b3[f, 0]
        b2p[s * 41 + 40] = 1.0

    big2 = np.zeros((128, BIG2C), f16)
    wT = np.ascontiguousarray(w["weights"].astype(f32).T).astype(f16)  # [256,64]
    big2[:, WT0:WT0 + 64] = wT[:128]
    big2[:, WT0 + 64:WT0 + 128] = wT[128:]
    ffw1 = w["ff_w1"].astype(f16)            # [256, 120]
    big2[:, FFW10:FFW10 + 120] = ffw1[:128]
    big2[:, FFW10 + 120:FFW10 + 240] = ffw1[128:]
    big2[:120, FFW20:FFW20 + 240] = w["ff_w2"].astype(f16)
    ffw3 = w["ff_w3"].astype(f16)            # [240, 400]
    big2[:120, FFW30:FFW30 + 400] = ffw3[:120, :]
    big2[:120, FFW30 + 800:FFW30 + 1200] = ffw3[120:, :]
    big2[:, IDN0:IDN0 + 128] = np.eye(128, dtype=f16)

    bias = np.zeros((128, 9), f32)
    bias[:120, 0] = b1p
    bias[:123, 1] = b2p
    bias[:120, 2] = w["ff_b1"].astype(f32)
    bias[:120, 3] = w["ff_b2"].astype(f32)[:120]
    bias[:120, 4] = w["ff_b2"].astype(f32)[120:240]
    ffb3 = np.zeros((512,), f32)
    ffb3[:K] = w["ff_b3"].astype(f32)
    for ci in range(4):
        bias[:, 5 + ci] = ffb3[128 * ci:128 * (ci + 1)]

    return w1p.astype(f16), w2p.astype(f16), w3p.astype(f16), big2, bias


def _pack_core(grid_c, cx, cy, inside_c):
    """Per-core rhs + scatter-index packing from the exact host mask."""
    f16 = np.float16
    rhs = np.full((38, NCH * W), 0.075, np.float32)
    sidx = np.full((128, 4 * J), -1, np.int16)
    for n in range(NLOC):
        ks = np.nonzero(inside_c[n])[0]
        assert len(ks) <= J
        g = n // G
        t_g, s_g = SLOT_OF_GROUP[g]
        m = n % G
        nt, prt = n // 128, n % 128
        cols = t_g * W + m * J + np.arange(len(ks))
        rhs[2 * s_g + 0, cols] = grid_c[n, 0] - cx[ks]   # exact fp32
        rhs[2 * s_g + 1, cols] = grid_c[n, 1] - cy[ks]
        base = 2 * J * nt
        sidx[prt, base:base + len(ks)] = ks              # field 0 -> col k
        sidx[prt, base + J:base + J + len(ks)] = 512 + ks  # field 1
    rhs[32:38, :] = rhs[0:6, :]   # dual L1 strip
    return rhs.astype(f16), sidx


def kernel(**inputs):
    global LAST_RESULTS
    nc = _get_nc()
    f32 = np.float32
    w1p, w2p, w3p, big2, bias = _pack_shared(inputs)

    grid = inputs["grid"].astype(f32)
    g1 = (np.arange(20, dtype=f32) * f32(0.05)).astype(f32)
    cx, cy = np.repeat(g1, 20), np.tile(g1, 20)
    centers = np.stack([cx, cy], -1)
    local = grid[:, None, :] - centers[None, :, :]
    inside = ((local >= 0) & (local <= f32(FILT))).all(-1)   # exact fp32 mask

    in_maps = []
    for c in range(NCORES):
        rhs, sidx = _pack_core(grid[c * NLOC:(c + 1) * NLOC], cx, cy,
                               inside[c * NLOC:(c + 1) * NLOC])
        big1 = np.zeros((128, BIG1C), np.float16)
        big1[:38, RHS0:RHS0 + NCH * W] = rhs
        big1[:38, W1P0:W1P0 + 120] = w1p
        big1[:120, W2P0:W2P0 + 123] = w2p
        big1[:123, W3P0:W3P0 + 32] = w3p
        in_maps.append(dict(big1=big1, big2=big2, sidx=sidx, bias=bias))

    res = run_bass_kernel_spmd(nc, in_maps, core_ids=list(range(NCORES)))
    LAST_RESULTS = res
    # device out is [B, F, NLOC]; harness wants [B, N, F]
    out = np.concatenate([r["out"].transpose(0, 2, 1) for r in res.results],
                         axis=1)
    return out
